# revision 17
# baseline (speedup 1.0000x reference)
"""Trainium2 Bass kernel for nn_DiagonalSSM (4-layer diagonal-SSM LM).

Sharding (8 cores):
  - Trunk: token-sharded. Core k handles batch k//4, tokens
    [(k%4)*512, (k%4+1)*512). The SSM recurrence runs as a native
    tensor_tensor_scan (one per 128-channel block, on DVE + Pool in
    parallel); cross-segment carries use one tiny AllGather (2 groups of 4)
    per layer that overlaps the local C-projection.
  - Head: token-sharded. Each core computes logits for its OWN 512 tokens
    over the FULL 32000-vocab.

Key structure (v3):
  - All layer-norm weight/bias folds are done on the host (n1w/n1b into the
    B-projection, n2w/n2b into w1/w2 + bias columns, now/nob into the head
    weights + bias). Device LN work is just stats + a 2-op normalize.
  - The Bu projection runs directly on h in [s, t] layout (moving dim =
    512 tokens, so every matmul is a full-rate 512-row pass); the
    -mu*cwB + Bb LN correction is applied per s-block with per-partition
    scalars in 3 fused vector ops.
  - The SSM scan h[t] = lam*h[t-1] + Bu[t] is one tensor_tensor_scan per
    s-block - no chunking, no carry fixups, no transposes.
  - The cross-core carry is accumulated into the C-projection PSUM group
    *after* the local matmuls, so the local C-projection overlaps the
    AllGather and only 8 matmuls wait on it.
  - w1/w2/w3 and the head run in bf16; per-layer tables and w1/w2 are
    double-buffered so next-layer weight DMA overlaps compute.
  - Transposes stream a bf16 identity (2x faster, numerically exact).
"""

import numpy as np

L, D, S, V = 4, 512, 256, 32000
DFF = 1368
B, T = 2, 2048
NCORES = 8
NSEG = 4
TSEG = 512
NVB = (V + 511) // 512          # 63 head vocab blocks (62 full + 256)
NVT = V // 128                  # 250 head vocab tiles
EPS = 1e-5
CH = 128
NCH = TSEG // CH   # 4
ND = D // 128      # 4
NS = S // 128      # 2
NFT = (DFF + 127) // 128  # 11
FTS = [128] * (DFF // 128) + ([DFF % 128] if DFF % 128 else [])
NLC = 2 * L        # packed LN-param columns: Dpw(L), Dpb(L)
HB = TSEG // 2

_NC_CACHE = {}


def _build_nc(debug=False):
    import concourse.bass as bass
    import concourse.tile as tile
    from concourse import bacc, mybir
    from concourse.masks import make_identity

    f32 = mybir.dt.float32
    f32r = mybir.dt.float32r
    bf16 = mybir.dt.bfloat16
    i32 = mybir.dt.int32
    AF = mybir.ActivationFunctionType
    OP = mybir.AluOpType

    nc = bacc.Bacc("TRN2", target_bir_lowering=False, debug=False,
                   num_devices=NCORES)

    # ---------------- DRAM I/O ----------------
    d_x = nc.dram_tensor("x_seg", [TSEG, 1], i32, kind="ExternalInput")
    d_emb = nc.dram_tensor("emb", [V, D], f32, kind="ExternalInput")
    d_pos = nc.dram_tensor("pos_seg", [TSEG, D], f32, kind="ExternalInput")
    d_BwTw = nc.dram_tensor("BwTw", [L, D, S], f32r, kind="ExternalInput")
    d_CwT = nc.dram_tensor("CwT", [L, S, D], f32r, kind="ExternalInput")
    d_w12 = nc.dram_tensor("w12n", [L, D, 2 * DFF], bf16, kind="ExternalInput")
    d_w3T = nc.dram_tensor("w3T", [L, DFF, D], bf16, kind="ExternalInput")
    d_cbc = nc.dram_tensor("cbc", [L, 128, 2 * NFT], f32, kind="ExternalInput")
    d_hW8 = nc.dram_tensor("hW8", [128, 2 * 4 * V], mybir.dt.float8e4,
                           kind="ExternalInput")
    d_cbH = nc.dram_tensor("cbH", [128, NVT], f32, kind="ExternalInput")
    d_lnc = nc.dram_tensor("lncols", [D, NLC], f32, kind="ExternalInput")
    # bcol: [lam | cwB | Bb] as per-channel columns
    d_bcol = nc.dram_tensor("bcol", [L, S, 3], f32, kind="ExternalInput")
    d_segcT = nc.dram_tensor("segcT", [L, S, NCH], f32, kind="ExternalInput")
    d_chc2T = nc.dram_tensor("chc2T", [L, S, TSEG], bf16,
                             kind="ExternalInput")
    d_onesD = nc.dram_tensor("onesD", [128, 128], f32r, kind="ExternalInput")

    d_out = nc.dram_tensor("logits", [V, TSEG], bf16, kind="ExternalOutput")

    with tile.TileContext(nc) as tc:
        with (
            tc.tile_pool(name="const", bufs=1) as cpool,
            tc.tile_pool(name="hm", bufs=1) as hm,
            tc.tile_pool(name="scr", bufs=2) as scr,
            tc.tile_pool(name="one", bufs=1) as one,
            tc.tile_pool(name="ps", bufs=1, space="PSUM") as ps,
            tc.tile_pool(name="ps4", bufs=4, space="PSUM") as ps4,
            tc.tile_pool(name="dram", bufs=2, space="DRAM") as dram,
            tc.tile_pool(name="tabs", bufs=2) as tabs,
            tc.tile_pool(name="wbig", bufs=2) as wbig,
        ):
            # ---------------- constants ----------------
            identb = cpool.tile([128, 128], f32, name="identb")
            make_identity(nc, identb[:, :])
            onesD_sb = cpool.tile([128, 128], f32r, name="onesD_sb")
            nc.sync.dma_start(out=onesD_sb[:, :], in_=d_onesD[:, :])
            eps_sb = cpool.tile([128, 1], f32, name="eps_sb")
            nc.vector.memset(eps_sb[:, :], EPS)
            onesT = cpool.tile([128, TSEG], f32, name="onesT")
            nc.gpsimd.memset(onesT[:, :], 1.0)
            lnc_sb = []
            for dd in range(ND):
                t = cpool.tile([128, NLC], f32, tag=f"lnc{dd}", name=f"lnc{dd}")
                nc.sync.dma_start(
                    out=t[:, :], in_=d_lnc[dd * 128:(dd + 1) * 128, :])
                lnc_sb.append(t)
            cbc_sb = [cpool.tile([128, 2 * NFT], f32, tag=f"cbc{ll}",
                                 name=f"cbc{ll}") for ll in range(L)]
            for ll in range(L):
                nc.scalar.dma_start(out=cbc_sb[ll][:, :], in_=d_cbc[ll])

            def lncol(key, ll, dd):
                c = {"Dpw": 0, "Dpb": L}[key] + ll
                return lnc_sb[dd][:, c:c + 1]

            # ---------------- h master (feature-major, f32r) --------------
            h = [hm.tile([128, TSEG], f32r, tag=f"h{dd}", name=f"h{dd}")
                 for dd in range(ND)]
            act_ctx = tc.tile_pool(name="act", bufs=1)
            act = act_ctx.__enter__()

            ENG = [nc.vector, nc.gpsimd]

            def cp_engine(i):
                # spread PSUM->SBUF copies across DVE / ACT (Pool can't
                # touch PSUM)
                if i % 2 == 1:
                    return nc.scalar.copy
                return nc.vector.tensor_copy

            # ---------------- per-layer table loads (double-buffered) ----
            def load_tables(ll):
                tb = {}
                tb["bcol"] = []
                tb["CwT"] = []
                tb["segcT"] = []
                tb["chc2T"] = []
                for ss in range(NS):
                    t = tabs.tile([128, 3], f32, tag=f"bcol{ss}",
                                  name=f"bcol{ll}_{ss}")
                    nc.scalar.dma_start(
                        out=t[:, :], in_=d_bcol[ll, ss * 128:(ss + 1) * 128, :])
                    tb["bcol"].append(t)
                tb["BwTw"] = []
                for dd in range(ND):
                    t = tabs.tile([128, S], f32r, tag=f"BwTw{dd}",
                                  name=f"BwTw{ll}_{dd}")
                    nc.scalar.dma_start(
                        out=t[:, :], in_=d_BwTw[ll, dd * 128:(dd + 1) * 128, :])
                    tb["BwTw"].append(t)
                for ss in range(NS):
                    t = tabs.tile([128, D], f32r, tag=f"CwT{ss}",
                                  name=f"CwT{ll}_{ss}")
                    nc.scalar.dma_start(
                        out=t[:, :], in_=d_CwT[ll, ss * 128:(ss + 1) * 128, :])
                    tb["CwT"].append(t)
                    t = tabs.tile([128, NCH], f32, tag=f"segcT{ss}",
                                  name=f"segcT{ll}_{ss}")
                    nc.scalar.dma_start(
                        out=t[:, :],
                        in_=d_segcT[ll, ss * 128:(ss + 1) * 128, :])
                    tb["segcT"].append(t)
                    t = tabs.tile([128, TSEG], bf16, tag=f"chc2T{ss}",
                                  name=f"chc2T{ll}_{ss}")
                    nc.scalar.dma_start(
                        out=t[:, :],
                        in_=d_chc2T[ll, ss * 128:(ss + 1) * 128, :])
                    tb["chc2T"].append(t)
                tb["w12"] = []
                for dd in range(ND):
                    t = wbig.tile([128, 2 * DFF], bf16, tag=f"w12_{dd}",
                                  name=f"w12_{ll}_{dd}")
                    nc.scalar.dma_start(
                        out=t[:, :], in_=d_w12[ll, dd * 128:(dd + 1) * 128, :])
                    tb["w12"].append(t)
                return tb

            tables = {0: load_tables(0)}

            # ---------------- embedding ----------------
            for tt_ in range(NCH):
                idx_sb = scr.tile([128, 1], i32, tag="idx", name="idx")
                nc.sync.dma_start(
                    out=idx_sb[:, :], in_=d_x[tt_ * 128:(tt_ + 1) * 128, :])
                e_t = one.tile([128, D], f32, tag="e_t", name="e_t", bufs=2)
                nc.gpsimd.indirect_dma_start(
                    out=e_t[:, :], out_offset=None, in_=d_emb[:, :],
                    in_offset=bass.IndirectOffsetOnAxis(ap=idx_sb[:, :1], axis=0),
                )
                p_t = one.tile([128, D], f32, tag="p_t", name="p_t")
                nc.sync.dma_start(
                    out=p_t[:, :], in_=d_pos[tt_ * 128:(tt_ + 1) * 128, :])
                htm = one.tile([128, D], f32, tag="htm", name="htm")
                ENG[tt_ % 2].tensor_tensor(
                    out=htm[:, :], in0=e_t[:, :], in1=p_t[:, :], op=OP.add)
                for dd in range(ND):
                    trp = ps.tile([128, 128], f32,
                                  tag=("pa" if dd % 2 == 0 else "pb"),
                                  name="trp")
                    nc.tensor.transpose(
                        trp[:, :], htm[:, dd * 128:(dd + 1) * 128],
                        identb[:, :])
                    cp_engine(dd)(
                        out=h[dd][:, tt_ * 128:(tt_ + 1) * 128], in_=trp[:, :])

            # ---- LN stats + chain: returns (mu_sb, rstd) bcast rows ----
            def ln_stats(nm):
                mu_ps = ps.tile([128, TSEG], f32, tag="sm", name=f"mu_{nm}")
                ex2_ps = ps.tile([128, TSEG], f32, tag="gps", name=f"ex2_{nm}")
                for dd in range(ND):
                    nc.tensor.matmul(mu_ps[:, :], onesD_sb[:, :], h[dd][:, :],
                                     start=(dd == 0), stop=(dd == ND - 1))
                for dd in range(ND):
                    h2 = scr.tile([128, TSEG], f32r, tag="h2", name="h2")
                    ENG[dd % 2].tensor_tensor(
                        out=h2[:, :], in0=h[dd][:, :], in1=h[dd][:, :],
                        op=OP.mult)
                    nc.tensor.matmul(ex2_ps[:, :], onesD_sb[:, :], h2[:, :],
                                     start=(dd == 0), stop=(dd == ND - 1))
                mu_sb = one.tile([128, TSEG], f32r, tag="mu_sb", name="mu_sb")
                nc.vector.tensor_copy(out=mu_sb[:, :], in_=mu_ps[:, :])
                musq = scr.tile([128, TSEG], f32, tag="musq", name="musq",
                                bufs=1)
                nc.gpsimd.tensor_tensor(
                    out=musq[:, :], in0=mu_sb[:, :], in1=mu_sb[:, :],
                    op=OP.mult)
                var = scr.tile([128, TSEG], f32, tag="var", name="var",
                               bufs=1)
                nc.vector.tensor_tensor(
                    out=var[:, :], in0=ex2_ps[:, :], in1=musq[:, :],
                    op=OP.subtract)
                sd = scr.tile([128, TSEG], f32, tag="sd", name="sd", bufs=1)
                nc.scalar.activation(out=sd[:, :], in_=var[:, :], func=AF.Sqrt,
                                     bias=eps_sb[:, :], scale=1.0)
                rstd = one.tile([128, TSEG], f32r, tag="rstd", name="rstd")
                with nc.allow_low_precision(reason="f32r rstd, 13-bit ok"):
                    nc.vector.reciprocal(out=rstd[:, :], in_=sd[:, :])
                return mu_sb, rstd

            # normalized bf16 output (LN2 / final LN), given stats
            def ln_norm_bf16(mu_sb, rstd, out_tag):
                xo = [act.tile([128, TSEG], bf16, tag=f"{out_tag}{dd}",
                               name=f"{out_tag}{dd}") for dd in range(ND)]
                for dd in range(ND):
                    for Hh in range(2):
                        cs = slice(Hh * HB, (Hh + 1) * HB)
                        eng = ENG[(dd + Hh) % 2]
                        hc = scr.tile([128, HB], f32, tag="hcb", name="hcb",
                                      bufs=4)
                        eng.tensor_tensor(
                            out=hc[:, :], in0=h[dd][:, cs], in1=mu_sb[:, cs],
                            op=OP.subtract)
                        eng.tensor_tensor(
                            out=xo[dd][:, cs], in0=hc[:, :], in1=rstd[:, cs],
                            op=OP.mult)
                return xo

            # ---------------- layers ----------------
            for ll in range(L):
                tb = tables.pop(ll)
                if ll + 1 < L:
                    tables[ll + 1] = load_tables(ll + 1)

                # ---- LN1 stats + Bu matmuls (start on raw h) ----
                mu_sb, rstd = ln_stats(f"l{ll}")

                bu_ps = []
                for ss in range(NS):
                    bp = ps.tile([128, TSEG], f32,
                                 tag=("pa" if ss == 0 else "pb"),
                                 name="bu_ps")
                    for dd in range(ND):
                        nc.tensor.matmul(
                            bp[:, :],
                            tb["BwTw"][dd][:, ss * 128:(ss + 1) * 128],
                            h[dd][:, :],
                            start=(dd == 0), stop=(dd == ND - 1))
                    bu_ps.append(bp)

                q_bc = one.tile([128, TSEG], f32, tag="q_bc", name="q_bc")
                nc.gpsimd.tensor_tensor(
                    out=q_bc[:, :], in0=mu_sb[:, :], in1=rstd[:, :],
                    op=OP.mult)

                # lam broadcast rows for the scan
                lam_bc = []
                for ss in range(NS):
                    lb = act.tile([128, TSEG], f32, tag=f"lam_bc{ss}",
                                  name=f"lam_bc{ss}")
                    ENG[ss].tensor_scalar_mul(
                        out=lb[:, :], in0=onesT[:, :],
                        scalar1=tb["bcol"][ss][:, 0:1])
                    lam_bc.append(lb)

                # ---- Bu epilogue + SSM scan (per s-block, DVE/Pool) ----
                hsT = []
                for ss in range(NS):
                    eng = ENG[ss]
                    m = scr.tile([128, TSEG], f32, tag="mrow", name="mrow",
                                 bufs=2)
                    nc.gpsimd.tensor_scalar(
                        out=m[:, :], in0=q_bc[:, :],
                        scalar1=tb["bcol"][ss][:, 1:2],
                        scalar2=tb["bcol"][ss][:, 2:3],
                        op0=OP.mult, op1=OP.add)
                    t = scr.tile([128, TSEG], f32, tag="but", name="but",
                                 bufs=2)
                    nc.vector.tensor_tensor(
                        out=t[:, :], in0=bu_ps[ss][:, :], in1=rstd[:, :],
                        op=OP.mult)
                    bu = scr.tile([128, TSEG], f32, tag="bu", name="bu",
                                  bufs=2)
                    nc.gpsimd.tensor_tensor(
                        out=bu[:, :], in0=t[:, :], in1=m[:, :],
                        op=OP.subtract)
                    hs = act.tile([128, TSEG], f32r, tag=f"hsT{ss}",
                                  name=f"hsT{ss}")
                    with nc.allow_low_precision(reason="scan state fp32"):
                        nc.vector.tensor_tensor_scan(
                            out=hs[:, :], data0=lam_bc[ss][:, :],
                            data1=bu[:, :], initial=0.0,
                            op0=OP.mult, op1=OP.add)
                    hsT.append(hs)

                # ---- G = final state; launch AllGather ASAP ----
                g_in = dram.tile([128, NS], f32r, tag="g_in", name="g_in")
                g_out = dram.tile([NCH, 128 * NS], f32r, tag="g_out",
                                  name="g_out")
                for ss in range(NS):
                    nc.sync.dma_start(
                        out=g_in[:, ss:ss + 1],
                        in_=hsT[ss][:, TSEG - 1:TSEG])
                nc.gpsimd.collective_compute(
                    "AllGather", mybir.AluOpType.bypass,
                    replica_groups=[[0, 1, 2, 3], [4, 5, 6, 7]],
                    ins=[g_in.opt()], outs=[g_out.opt()],
                )

                # ---- du (D-term) - independent of the collective ----
                du = []
                for dd in range(ND):
                    eng = ENG[dd % 2]
                    hc = scr.tile([128, TSEG], f32, tag="hc", name="hc",
                                  bufs=2)
                    eng.tensor_tensor(
                        out=hc[:, :], in0=h[dd][:, :], in1=mu_sb[:, :],
                        op=OP.subtract)
                    t2 = scr.tile([128, TSEG], f32, tag="t2", name="t2",
                                  bufs=2)
                    eng.tensor_tensor(
                        out=t2[:, :], in0=hc[:, :], in1=rstd[:, :],
                        op=OP.mult)
                    dut = act.tile([128, TSEG], bf16, tag=f"du{dd}",
                                   name=f"du{dd}")
                    eng.tensor_scalar(
                        out=dut[:, :], in0=t2[:, :],
                        scalar1=lncol("Dpw", ll, dd),
                        scalar2=lncol("Dpb", ll, dd),
                        op0=OP.mult, op1=OP.add)
                    du.append(dut)

                # ---- local C-projection (overlaps the collective) ----
                cp_ps = []
                for dd in range(ND):
                    cp = ps4.tile([128, TSEG], f32, tag="acc", name="cp_ps")
                    for ss in range(NS):
                        nc.tensor.matmul(
                            cp[:, :],
                            tb["CwT"][ss][:, dd * 128:(dd + 1) * 128],
                            hsT[ss][:, :],
                            start=(ss == 0), stop=False)
                    cp_ps.append(cp)

                # ---- cross-core carry: E from AllGather, accumulate ----
                CwET = []
                for ss in range(NS):
                    t = scr.tile([128, NCH], f32r, tag="GallT", name="GallT",
                                 bufs=2)
                    g_ap = g_out[:, :]
                    nc.sync.dma_start(
                        out=t[:, :],
                        in_=bass.AP(tensor=g_ap.tensor,
                                    offset=g_ap.offset + ss,
                                    ap=[[NS, 128], [128 * NS, NCH]]))
                    prod = scr.tile([128, NCH], f32, tag="prodE", name="prodE",
                                    bufs=2)
                    nc.vector.tensor_tensor(
                        out=prod[:, :], in0=t[:, :],
                        in1=tb["segcT"][ss][:, :], op=OP.mult)
                    e01 = scr.tile([128, 1], f32, tag="e01", name="e01",
                                   bufs=2)
                    nc.vector.tensor_tensor(
                        out=e01[:, :], in0=prod[:, 0:1], in1=prod[:, 1:2],
                        op=OP.add)
                    e23 = scr.tile([128, 1], f32, tag="e23", name="e23",
                                   bufs=2)
                    nc.vector.tensor_tensor(
                        out=e23[:, :], in0=prod[:, 2:3], in1=prod[:, 3:4],
                        op=OP.add)
                    ecol = scr.tile([128, 1], f32, tag="ecol", name="ecol",
                                    bufs=2)
                    nc.vector.tensor_tensor(
                        out=ecol[:, :], in0=e01[:, :], in1=e23[:, :],
                        op=OP.add)
                    cwe = scr.tile([128, D], bf16, tag="cwe", name="cwe",
                                   bufs=2)
                    ENG[ss % 2].tensor_scalar_mul(
                        out=cwe[:, :], in0=tb["CwT"][ss][:, :],
                        scalar1=ecol[:, :])
                    CwET.append(cwe)
                for dd in range(ND):
                    for ss in range(NS):
                        nc.tensor.matmul(
                            cp_ps[dd][:, :],
                            CwET[ss][:, dd * 128:(dd + 1) * 128],
                            tb["chc2T"][ss][:, :],
                            start=False, stop=(ss == NS - 1))

                # ---- h += ssm_out (DVE reads PSUM, Pool adds into h) --
                for dd in range(ND):
                    for Hh in range(2):
                        cs = slice(Hh * HB, (Hh + 1) * HB)
                        u = scr.tile([128, HB], f32, tag="updu", name="updu",
                                     bufs=4)
                        nc.vector.tensor_tensor(
                            out=u[:, :], in0=cp_ps[dd][:, cs],
                            in1=du[dd][:, cs], op=OP.add)
                        nc.gpsimd.tensor_tensor(
                            out=h[dd][:, cs], in0=h[dd][:, cs],
                            in1=u[:, :], op=OP.add)

                # ---- LN2 + SwiGLU ----
                mu2, rstd2 = ln_stats(f"m{ll}")
                t1 = ln_norm_bf16(mu2, rstd2, "xm")
                sw_ps = [ps4.tile([128, TSEG], f32, tag="acc", name="sw_ps")
                         for _ in range(ND)]
                f0 = 0
                for fi, pf in enumerate(FTS):
                    a_ps = ps.tile([128, TSEG], f32, tag="pa", name="a_ps")
                    b_ps = ps.tile([128, TSEG], f32, tag="pb", name="b_ps")
                    for dd in range(ND):
                        nc.tensor.matmul(
                            a_ps[:pf, :],
                            tb["w12"][dd][:, f0:f0 + pf],
                            t1[dd][:, :],
                            start=(dd == 0), stop=(dd == ND - 1))
                    for dd in range(ND):
                        nc.tensor.matmul(
                            b_ps[:pf, :],
                            tb["w12"][dd][:, DFF + f0:DFF + f0 + pf],
                            t1[dd][:, :],
                            start=(dd == 0), stop=(dd == ND - 1))
                    sa = scr.tile([128, TSEG], bf16, tag="sa", name="sa")
                    nc.scalar.activation(
                        out=sa[:pf, :], in_=a_ps[:pf, :], func=AF.Silu,
                        bias=cbc_sb[ll][:pf, fi:fi + 1], scale=1.0)
                    g = scr.tile([128, TSEG], bf16, tag="g", name="g")
                    nc.vector.scalar_tensor_tensor(
                        out=g[:pf, :], in0=b_ps[:pf, :],
                        scalar=cbc_sb[ll][:pf, NFT + fi:NFT + fi + 1],
                        in1=sa[:pf, :], op0=OP.add, op1=OP.mult)
                    w3t = scr.tile([128, D], bf16, tag="w3t", name="w3t",
                                   bufs=3)
                    nc.sync.dma_start(
                        out=w3t[:pf, :], in_=d_w3T[ll, f0:f0 + pf, :])
                    for dd in range(ND):
                        nc.tensor.matmul(
                            sw_ps[dd][:, :],
                            w3t[:pf, dd * 128:(dd + 1) * 128],
                            g[:pf, :],
                            start=(fi == 0), stop=(fi == NFT - 1))
                    f0 += pf
                for dd in range(ND):
                    swc = scr.tile([128, TSEG], f32, tag="swc", name="swc",
                                   bufs=2)
                    cp_engine(dd)(out=swc[:, :], in_=sw_ps[dd][:, :])
                    for Hh in range(2):
                        cs = slice(Hh * HB, (Hh + 1) * HB)
                        ENG[(dd + Hh) % 2].tensor_tensor(
                            out=h[dd][:, cs], in0=h[dd][:, cs],
                            in1=swc[:, cs], op=OP.add)

            # ---------------- final LN (bf16 out for the bf16 head) ------
            muf, rstdf = ln_stats("f")
            xnf = ln_norm_bf16(muf, rstdf, "xm")

            # ---------------- head (fp8 hi/lo 3-pass, token-sharded) -----
            f8 = mybir.dt.float8e4
            DR = mybir.MatmulPerfMode.DoubleRow
            with tc.tile_pool(name="hd", bufs=1) as hd, \
                 tc.tile_pool(name="hw2", bufs=3) as hw2, \
                 tc.tile_pool(name="ho", bufs=6) as ho:
                cbH_sb = hd.tile([128, NVT], f32, tag="cbH", name="cbH")
                nc.sync.dma_start(out=cbH_sb[:, :], in_=d_cbH[:, :])
                # xnf -> fp8 hi + lo planes, [128, dd, T] layout
                xq_hi = hd.tile([128, ND, TSEG], f8, tag="xq_hi", name="xq_hi")
                xq_lo = hd.tile([128, ND, TSEG], f8, tag="xq_lo", name="xq_lo")
                for dd in range(ND):
                    ENG[dd % 2].tensor_copy(
                        out=xq_hi[:, dd, :], in_=xnf[dd][:, :])
                    ENG[dd % 2].tensor_tensor(
                        out=xq_lo[:, dd, :], in0=xnf[dd][:, :],
                        in1=xq_hi[:, dd, :], op=OP.subtract)
                out_ap = d_out[:, :]
                hw_ap = d_hW8[:, :]
                for vb in range(NVB):
                    v0 = vb * 512
                    nv = min(512, V - v0)
                    nsub = nv // 128
                    wt = hw2.tile([128, 2, ND, 512], f8, tag="wt", name="wt")
                    wq = [nc.sync, nc.scalar][vb % 2]
                    wq.dma_start(
                        out=wt[:, :, :, :nv],
                        in_=bass.AP(tensor=hw_ap.tensor,
                                    offset=hw_ap.offset + v0,
                                    ap=[[8 * V, 128], [4 * V, 2], [V, 4],
                                        [1, nv]]))
                    for sub in range(nsub):
                        hp_ps = ps4.tile([128, TSEG], f32, tag="acc",
                                         name="hp_ps")
                        c0 = sub * 128
                        for pl, xq in ((0, xq_hi), (0, xq_lo), (1, xq_hi)):
                            for j in range(2):
                                nc.tensor.matmul(
                                    hp_ps[:, :],
                                    wt[:, pl, 2 * j:2 * j + 2, c0:c0 + 128],
                                    xq[:, 2 * j:2 * j + 2, :],
                                    start=(pl == 0 and xq is xq_hi
                                           and j == 0),
                                    stop=(pl == 1 and j == 1),
                                    perf_mode=DR)
                        bcol = cbH_sb[:, vb * 4 + sub:vb * 4 + sub + 1]
                        ot = ho.tile([128, TSEG], bf16, tag="ot", name="ot")
                        if (vb * 4 + sub) % 2 == 1:
                            nc.scalar.activation(
                                out=ot[:, :], in_=hp_ps[:, :],
                                func=AF.Identity, bias=bcol,
                                scale=1.0 / 64.0)
                        else:
                            nc.vector.tensor_scalar(
                                out=ot[:, :], in0=hp_ps[:, :],
                                scalar1=1.0 / 64.0, scalar2=bcol,
                                op0=OP.mult, op1=OP.add)
                        oq = [nc.scalar, nc.sync][(vb * 4 + sub) % 2]
                        oq.dma_start(
                            out=bass.AP(tensor=out_ap.tensor,
                                        offset=out_ap.offset
                                        + (v0 + sub * 128) * TSEG,
                                        ap=[[TSEG, 128], [1, TSEG]]),
                            in_=ot[:, :])

            act_ctx.__exit__(None, None, None)

    nc.compile()
    return nc


def _host_prep(inputs):
    """Build the 8 per-core input maps from full inputs."""
    import ml_dtypes

    x = np.asarray(inputs["x"]).astype(np.int32)
    emb = np.asarray(inputs["emb"], np.float32)
    pos = np.asarray(inputs["pos"], np.float32)
    lam = 1.0 / (1.0 + np.exp(-np.asarray(inputs["log_lambda"], np.float64)))
    Bw = np.asarray(inputs["Bw"], np.float32)      # [L, S, D]
    Cw = np.asarray(inputs["Cw"], np.float32)      # [L, D, S]
    w1 = np.asarray(inputs["w1"], np.float32)      # [L, DFF, D]
    w2 = np.asarray(inputs["w2"], np.float32)
    w3 = np.asarray(inputs["w3"], np.float32)      # [L, D, DFF]
    headW = np.asarray(inputs["headW"], np.float32)
    headb = np.asarray(inputs["headb"], np.float32)
    Dp = np.asarray(inputs["Dp"], np.float32)
    n1w = np.asarray(inputs["n1w"], np.float32)
    n1b = np.asarray(inputs["n1b"], np.float32)
    n2w = np.asarray(inputs["n2w"], np.float32)
    n2b = np.asarray(inputs["n2b"], np.float32)
    now = np.asarray(inputs["now"], np.float32)
    nob = np.asarray(inputs["nob"], np.float32)

    # ---- folded projections ----
    Bwn = Bw * n1w[:, None, :]                       # [L,S,D]
    BwTw = np.ascontiguousarray(Bwn.transpose(0, 2, 1))   # [L,D,S]
    cwB = Bwn.sum(-1)                                # [L,S]
    Bb = (Bw * n1b[:, None, :]).sum(-1)              # [L,S]
    CwT = np.ascontiguousarray(Cw.transpose(0, 2, 1))     # [L,S,D]

    bcol = np.stack([lam.astype(np.float32), cwB, -Bb], axis=-1)  # [L,S,3]
    bcol = np.ascontiguousarray(bcol)

    w1n = (w1 * n2w[:, None, :]).transpose(0, 2, 1)  # [L,D,DFF]
    w2n = (w2 * n2w[:, None, :]).transpose(0, 2, 1)
    w12n = np.ascontiguousarray(
        np.concatenate([w1n, w2n], axis=2)).astype(ml_dtypes.bfloat16)
    cb1 = np.einsum("lfd,ld->lf", w1, n2b)           # [L,DFF]
    cb2 = np.einsum("lfd,ld->lf", w2, n2b)
    cbc = np.zeros((L, 128, 2 * NFT), np.float32)
    pad = NFT * 128 - DFF
    cb1p = np.pad(cb1, ((0, 0), (0, pad)))
    cb2p = np.pad(cb2, ((0, 0), (0, pad)))
    cbc[:, :, :NFT] = cb1p.reshape(L, NFT, 128).transpose(0, 2, 1)
    cbc[:, :, NFT:] = cb2p.reshape(L, NFT, 128).transpose(0, 2, 1)
    w3T = np.ascontiguousarray(
        w3.transpose(0, 2, 1)).astype(ml_dtypes.bfloat16)

    # head folds: hWn = headW * now; cbH = headW@nob + headb
    # fp8 hi/lo planes at x64 scale: hW8[p, plane, dd, v]
    hWn = (headW * now[None, :]).astype(np.float64) * 64.0
    f8 = ml_dtypes.float8_e4m3
    Whi = hWn.astype(f8)
    Wlo = (hWn - Whi.astype(np.float64)).astype(f8)
    hW8 = np.empty((128, 2, ND, V), f8)
    for pl, Wq in ((0, Whi), (1, Wlo)):
        # Wq[v, d]; want [p, dd, v] with d = dd*128 + p
        hW8[:, pl] = Wq.T.reshape(ND, 128, V).transpose(1, 0, 2)
    hW8 = np.ascontiguousarray(hW8.reshape(128, 2 * 4 * V))
    cbH = headW @ nob + headb                        # [V]
    cbHc = np.ascontiguousarray(cbH.reshape(NVT, 128).T)  # [128, NVT]

    # packed LN params: [Dp*n1w (L), Dp*n1b (L)]
    lncols = np.zeros((D, NLC), np.float32)
    lncols[:, :L] = (Dp * n1w).T
    lncols[:, L:] = (Dp * n1b).T

    t_ar = np.arange(TSEG, dtype=np.float64)[None, None, :]
    chc2T = (lam[:, :, None] ** (t_ar + 1)).astype(
        ml_dtypes.bfloat16)  # [L,S,T]

    onesD = np.full((128, 128), 1.0 / D, np.float32)

    in_maps = []
    for k in range(NCORES):
        b, r = divmod(k, NSEG)
        t0 = r * TSEG
        segcT = np.zeros((L, S, NCH), np.float32)
        for sp in range(r):
            segcT[:, :, sp] = (lam ** (TSEG * (r - 1 - sp))).astype(
                np.float32)
        in_maps.append({
            "x_seg": np.ascontiguousarray(x[b, t0:t0 + TSEG, None]),
            "emb": emb,
            "pos_seg": np.ascontiguousarray(pos[t0:t0 + TSEG]),
            "BwTw": BwTw, "CwT": CwT,
            "w12n": w12n, "w3T": w3T, "cbc": cbc,
            "hW8": hW8, "cbH": cbHc,
            "lncols": lncols,
            "bcol": bcol,
            "segcT": np.ascontiguousarray(segcT),
            "chc2T": chc2T,
            "onesD": onesD,
        })
    return in_maps


def kernel(**inputs) -> np.ndarray:
    from concourse.bass_utils import run_bass_kernel_spmd

    if "nc" not in _NC_CACHE:
        _NC_CACHE["nc"] = _build_nc()
    nc = _NC_CACHE["nc"]
    in_maps = _host_prep(inputs)
    res = None
    last_err = None
    for _attempt in range(3):
        try:
            res = run_bass_kernel_spmd(nc, in_maps, core_ids=list(range(NCORES)))
            break
        except Exception as e:  # transient device hiccups: retry
            last_err = e
    if res is None:
        raise last_err
    return assemble_output([res.results[k]["logits"] for k in range(NCORES)])


def assemble_output(parts):
    """parts[k]: [V, TSEG] bf16 logits for core k's tokens -> [B, T, V] f32."""
    out = np.empty((B, T, V), np.float32)
    for k in range(NCORES):
        b, r = divmod(k, NSEG)
        out[b, r * TSEG:(r + 1) * TSEG] = parts[k].astype(np.float32).T
    return out
